# Initial kernel scaffold
#
"""GPT-OSS MoE experts kernel for Trainium2 (8 NeuronCores, expert-parallel).

Strategy
--------
- Expert-parallel: core e owns expert e's weights (1/8 of total weight bytes,
  read exactly once -> memory-bound). Host does routing (gather tokens per
  expert), weight re-staging (slice expert, transpose to contraction-major
  [K, N] tile layout, pad to multiples of 128, cast fp16), and the final
  scatter-add combine. No collectives needed.
- The reference's per-32-block fp8 quant-dequant collapses exactly to
  "round each element to 4 significant bits (RTNE)": the block scale is a
  power of two (mantissa rounding is scale-invariant) and the +-448 clip can
  never bind by construction. Verified numerically; residual differences are
  confined to the e4m3-subnormal range (~2^-9 * block scale, negligible).
  On device this is 3 VectorE ops (Veltkamp split); the 4-significant-bit
  activation values are then EXACT in fp16.
- fp16 weights round at 2^-11; end-to-end error vs the f32 reference is
  ~6e-3 absmax-rel (vs ~5e-3 with fp32 weights) - the budget is dominated by
  quantization-boundary flips from layer-1 perturbations either way, and
  fp16 halves the weight traffic of this DMA-bound kernel.
- Form-B matmuls: weight [128, 128] tiles are the STATIONARY operand (LDW
  pipelines at ~72 ns/pair with fast-weight-load), ALL tokens ride the
  moving free dim (N = padded token count <= 512). PE cost is independent
  of the token distribution, outputs land output-major ([n, tokens]), which
  feeds layer 2 directly - no on-chip transposes at all.
- Biases ride free inside the GEMM: contraction padded 2880 -> 2944, the
  activations carry a constant-1 row at index 2880, weights the bias row.
- Both weight matrices are staged n-tile-major on host, so every weight DMA
  is one fully-contiguous ~750 KB read.
"""

import functools
import sys

sys.path.insert(0, "/opt/trn_rl_repo")

import numpy as np

import concourse.bass as bass  # noqa: F401
import concourse.mybir as mybir
import concourse.tile as tile
from concourse import bacc
from concourse.bass_utils import run_bass_kernel_spmd

P = 128
H = 2880          # hidden dim
II = 2880         # intermediate dim (gate/up width)
NE = 8            # experts == cores
KT = 23           # 128-tiles over the padded contraction dim
KP = KT * P       # 2944 = 2880 + 64 (row 2880 = ones/bias row, rest zero)
NT = 23           # 128-tiles over the padded output dims (II, H -> 2944)
VC = float(2 ** 20 + 1)   # Veltkamp constant: RTNE to 4 significant bits
MAXTOK = 512              # moving free-dim (= PSUM f32 bank) limit

f32 = mybir.dt.float32
f16 = mybir.dt.float16
AF = mybir.ActivationFunctionType
ALU = mybir.AluOpType


def _rtne4(x):
    """Round f32 elements to 4 significant bits, RTNE (== reference
    quant_dequant_fp8 up to e4m3-subnormal leftovers)."""
    c = np.float32(VC)
    t = (x * c).astype(np.float32)
    return (t - (t - x)).astype(np.float32)


@functools.lru_cache(maxsize=4)
def _build(ccap):
    """Per-core Bass program; ccap = padded token capacity (<= MAXTOK)."""
    nc = bacc.Bacc(None, target_bir_lowering=False)

    xt_d = nc.declare_dram_parameter("xt", [P, KT, ccap], f16, isOutput=False)
    w1g_d = nc.declare_dram_parameter("w1g", [NT, P, KT, P], f16, isOutput=False)
    w1u_d = nc.declare_dram_parameter("w1u", [NT, P, KT, P], f16, isOutput=False)
    w2_d = nc.declare_dram_parameter("w2", [NT, P, KT, P], f16, isOutput=False)
    wr_d = nc.declare_dram_parameter("wr", [P, ccap], f32, isOutput=False)
    y_d = nc.declare_dram_parameter("y", [NT, P, ccap], f16, isOutput=True)

    with tile.TileContext(nc) as tc:
        with (
            tc.tile_pool(name="consts", bufs=1) as consts,
            tc.tile_pool(name="wslab", bufs=26) as wpool,
            tc.tile_pool(name="tmp", bufs=2) as tmp,
            tc.tile_pool(name="psum", bufs=4, space="PSUM") as psum,
        ):
            # resident tensors
            xts = consts.tile([P, KT, ccap], f16, tag="xt", name="xt")
            nc.sync.dma_start(xts, xt_d[:])
            wrep = consts.tile([P, ccap], f32, tag="wrep", name="wrep")
            nc.sync.dma_start(wrep, wr_d[:])
            interT = consts.tile([P, KT, ccap], f16, tag="interT", name="interT")

            # HAM warmup while the first slabs + xt stream in
            wtile = consts.tile([P, P], f16, tag="wtile", name="wtile")
            nc.vector.memset(wtile, 0.25)
            wup = psum.tile([P, ccap], f32, tag="ps_g", name="wup")
            for _ in range(48):
                nc.tensor.matmul(wup[:, :P], wtile, wtile,
                                 start=True, stop=True, skip_group_check=True)

            def load_slab(src, nt):
                s = wpool.tile([P, KT, P], f16, tag="wslab", name="wslab")
                nc.sync.dma_start(s, src[nt])
                return s

            # ---- layer 1 + swiglu + rtne4, one 128-wide n-tile at a time ----
            for nt in range(NT):
                slab_g = load_slab(w1g_d, nt)
                slab_u = load_slab(w1u_d, nt)
                gps = psum.tile([P, ccap], f32, tag="ps_g", name="ps_g")
                ups = psum.tile([P, ccap], f32, tag="ps_u", name="ps_u")
                for k in range(KT):
                    nc.tensor.matmul(gps, slab_g[:, k, :], xts[:, k, :],
                                     start=(k == 0), stop=(k == KT - 1))
                    nc.tensor.matmul(ups, slab_u[:, k, :], xts[:, k, :],
                                     start=(k == 0), stop=(k == KT - 1))
                # swiglu: gate=min(G,7); up1=clip(U,-7,7)+1; x=gate*sig(1.702g)*up1
                gate = tmp.tile([P, ccap], f32, tag="t_gate", name="t_gate")
                nc.vector.tensor_scalar_min(gate, gps, 7.0)
                sig = tmp.tile([P, ccap], f32, tag="t_sig", name="t_sig")
                nc.scalar.activation(sig, gate, AF.Sigmoid, scale=1.702)
                up1 = tmp.tile([P, ccap], f32, tag="t_up", name="t_up")
                nc.vector.tensor_scalar(up1, ups, 1.0, -6.0, ALU.add, ALU.max)
                nc.vector.tensor_scalar_min(up1, up1, 8.0)
                nc.vector.tensor_mul(gate, gate, sig)          # gate*sig
                xv = tmp.tile([P, ccap], f32, tag="t_xv", name="t_xv")
                nc.vector.tensor_mul(xv, gate, up1)            # x = swiglu
                tv = tmp.tile([P, ccap], f32, tag="t_tv", name="t_tv")
                nc.vector.tensor_scalar_mul(tv, xv, VC)        # t = x*c
                nc.vector.tensor_sub(xv, tv, xv)               # d = t-x
                nc.vector.tensor_sub(interT[:, nt, :], tv, xv)  # rtne4 = t-d
            # layer-2 bias row: i == II lives at tile KT-1, partition II % P
            nc.vector.memset(interT[II % P : II % P + 1, KT - 1, :], 1.0)

            # ---- layer 2 + routing-weight scale (y stores batched 6-wide) ----
            YB = 6
            ysb = None
            for ht in range(NT):
                slab2 = load_slab(w2_d, ht)
                yps = psum.tile([P, ccap], f32, tag="ps_g", name="ps_g")
                for k in range(KT):
                    nc.tensor.matmul(yps, slab2[:, k, :], interT[:, k, :],
                                     start=(k == 0), stop=(k == KT - 1))
                if ht % YB == 0:
                    nyb = min(YB, NT - ht)
                    ysb = tmp.tile([P, YB, ccap], f16, tag="ysb", name="ysb")
                nc.vector.tensor_mul(ysb[:, ht % YB, :], yps, wrep)
                if ht % YB == nyb - 1:
                    nc.scalar.dma_start(
                        y_d[ht - nyb + 1 : ht + 1].rearrange("t p c -> p t c"),
                        ysb[:, :nyb, :],
                    )

    nc.finalize()
    return nc


def _stage(inputs):
    """Host-side routing + weight re-staging. Returns (nc, in_maps, assigns, T)."""
    hs = np.ascontiguousarray(np.asarray(inputs["hidden_states"], dtype=np.float32))
    ri = np.asarray(inputs["router_indices"]).astype(np.int64)
    rw = np.asarray(inputs["routing_weights"], dtype=np.float32)
    gup = np.asarray(inputs["gate_up_proj"], dtype=np.float32)
    gub = np.asarray(inputs["gate_up_proj_bias"], dtype=np.float32)
    dn = np.asarray(inputs["down_proj"], dtype=np.float32)
    dnb = np.asarray(inputs["down_proj_bias"], dtype=np.float32)

    T = hs.shape[0]
    topk = ri.shape[1]

    flat_e = ri.reshape(-1)
    order = np.argsort(flat_e, kind="stable")
    counts = np.bincount(flat_e, minlength=NE)
    starts = np.zeros(NE + 1, np.int64)
    starts[1:] = np.cumsum(counts)
    maxc = int(counts.max())
    # Each pass handles up to MAXTOK tokens per expert (seed-0 loads are ~142,
    # so this is a single pass; multiple passes only for pathological routing).
    npass = max(1, -(-maxc // MAXTOK))
    percap = -(-maxc // npass)
    ccap = max(32, -(-percap // 32) * 32)

    x_dq = _rtne4(hs).astype(np.float16)   # 4-sig-bit values: exact in fp16
    rw_flat = rw.reshape(-1)

    def stage_w(mat_t, bias):
        # mat_t: [K <= 2880, N <= 2880] contraction-major; -> tiled layout
        w = np.zeros((KP, NT * P), np.float16)
        w[: mat_t.shape[0], : mat_t.shape[1]] = mat_t
        w[H, : bias.shape[0]] = bias
        # [NT, P(k-partition), KT, P(n)] so each partition's slab bytes are
        # one contiguous HBM run
        return np.ascontiguousarray(
            w.reshape(KT, P, NT, P).transpose(2, 1, 0, 3)
        )

    passes, assigns = [], []
    weights = []
    for e in range(NE):
        weights.append((
            stage_w(gup[e, 0::2, :].T.astype(np.float16), gub[e, 0::2]),
            stage_w(gup[e, 1::2, :].T.astype(np.float16), gub[e, 1::2]),
            stage_w(dn[e].T.astype(np.float16), dnb[e]),
        ))
    for p in range(npass):
        in_maps, passigns = [], []
        for e in range(NE):
            a_all = order[starts[e] : starts[e + 1]]
            a = a_all[p * ccap : (p + 1) * ccap]
            toks = a // topk
            ce = len(a)
            passigns.append((a, toks))

            xt = np.zeros((KP, ccap), np.float16)
            xt[:H, :ce] = x_dq[toks].T
            xt[H, :] = np.float16(1.0)
            xt = np.ascontiguousarray(xt.reshape(KT, P, ccap).transpose(1, 0, 2))

            wr_rep = np.zeros((P, ccap), np.float32)
            wr_rep[:, :ce] = rw_flat[a][None, :]

            w1g, w1u, w2 = weights[e]
            in_maps.append(dict(xt=xt, w1g=w1g, w1u=w1u, w2=w2, wr=wr_rep))
        passes.append(in_maps)
        assigns.append(passigns)

    nc = _build(ccap)
    return nc, passes, assigns, T


def kernel(**inputs):
    nc, passes, assigns, T = _stage(inputs)
    out = np.zeros((T, H), np.float32)
    for in_maps, passigns in zip(passes, assigns):
        res = run_bass_kernel_spmd(nc, in_maps, list(range(NE)))
        for e in range(NE):
            a, toks = passigns[e]
            if len(a):
                yt = res.results[e]["y"].reshape(NT * P, -1)[:H, : len(a)]
                np.add.at(out, toks, yt.T.astype(np.float32))
    return out



# revision 1
# speedup vs baseline: 1.0855x; 1.0855x over previous
"""GPT-OSS MoE experts kernel for Trainium2 (8 NeuronCores, expert-parallel).

Strategy
--------
- Expert-parallel: core e owns expert e's weights (1/8 of total weight bytes,
  read exactly once -> memory-bound). Host does routing (gather tokens per
  expert), weight re-staging (slice expert, transpose to contraction-major
  [K, N] tile layout, pad to multiples of 128, cast fp16), and the final
  scatter-add combine. No collectives needed.
- The reference's per-32-block fp8 quant-dequant collapses exactly to
  "round each element to 4 significant bits (RTNE)": the block scale is a
  power of two (mantissa rounding is scale-invariant) and the +-448 clip can
  never bind by construction. Verified numerically; residual differences are
  confined to the e4m3-subnormal range (~2^-9 * block scale, negligible).
  On device this is 3 VectorE ops (Veltkamp split); the 4-significant-bit
  activation values are then EXACT in fp16.
- fp16 weights round at 2^-11; end-to-end error vs the f32 reference is
  ~6e-3 absmax-rel (vs ~5e-3 with fp32 weights) - the budget is dominated by
  quantization-boundary flips from layer-1 perturbations either way, and
  fp16 halves the weight traffic of this DMA-bound kernel.
- Form-B matmuls: weight [128, 128] tiles are the STATIONARY operand (LDW
  pipelines at ~72 ns/pair with fast-weight-load), ALL tokens ride the
  moving free dim (N = padded token count <= 512). PE cost is independent
  of the token distribution, outputs land output-major ([n, tokens]), which
  feeds layer 2 directly - no on-chip transposes at all.
- Biases ride free inside the GEMM: contraction padded 2880 -> 2944, the
  activations carry a constant-1 row at index 2880, weights the bias row.
- Both weight matrices are staged n-tile-major on host, so every weight DMA
  is one fully-contiguous ~750 KB read.
"""

import functools
import sys

sys.path.insert(0, "/opt/trn_rl_repo")

import numpy as np

import concourse.bass as bass  # noqa: F401
import concourse.mybir as mybir
import concourse.tile as tile
from concourse import bacc
from concourse.bass_utils import run_bass_kernel_spmd

P = 128
H = 2880          # hidden dim
II = 2880         # intermediate dim (gate/up width)
NE = 8            # experts == cores
KT = 23           # 128-tiles over the padded contraction dim
KP = KT * P       # 2944 = 2880 + 64 (row 2880 = ones/bias row, rest zero)
NT = 23           # 128-tiles over the padded output dims (II, H -> 2944)
VC = float(2 ** 20 + 1)   # Veltkamp constant: RTNE to 4 significant bits
MAXTOK = 512              # moving free-dim (= PSUM f32 bank) limit

f32 = mybir.dt.float32
f16 = mybir.dt.float16
AF = mybir.ActivationFunctionType
ALU = mybir.AluOpType


def _rtne4(x):
    """Round f32 elements to 4 significant bits, RTNE (== reference
    quant_dequant_fp8 up to e4m3-subnormal leftovers)."""
    c = np.float32(VC)
    t = (x * c).astype(np.float32)
    return (t - (t - x)).astype(np.float32)


@functools.lru_cache(maxsize=4)
def _build(ccap):
    """Per-core Bass program; ccap = padded token capacity (<= MAXTOK)."""
    nc = bacc.Bacc(None, target_bir_lowering=False)

    xt_d = nc.declare_dram_parameter("xt", [P, KT, ccap], f16, isOutput=False)
    w1g_d = nc.declare_dram_parameter("w1g", [NT, P, KT, P], f16, isOutput=False)
    w1u_d = nc.declare_dram_parameter("w1u", [NT, P, KT, P], f16, isOutput=False)
    w2_d = nc.declare_dram_parameter("w2", [NT, P, KT, P], f16, isOutput=False)
    wr_d = nc.declare_dram_parameter("wr", [P, ccap], f32, isOutput=False)
    y_d = nc.declare_dram_parameter("y", [NT, P, ccap], f16, isOutput=True)

    with tile.TileContext(nc) as tc:
        with (
            tc.tile_pool(name="consts", bufs=1) as consts,
            tc.tile_pool(name="wslab", bufs=26) as wpool,
            tc.tile_pool(name="tmp", bufs=2) as tmp,
            tc.tile_pool(name="psum", bufs=4, space="PSUM") as psum,
        ):
            # resident tensors
            xts = consts.tile([P, KT, ccap], f16, tag="xt", name="xt")
            nc.sync.dma_start(xts, xt_d[:])
            wrep = consts.tile([P, ccap], f32, tag="wrep", name="wrep")
            nc.sync.dma_start(wrep, wr_d[:])
            interT = consts.tile([P, KT, ccap], f16, tag="interT", name="interT")

            # HAM warmup while the first slabs + xt stream in
            wtile = consts.tile([P, P], f16, tag="wtile", name="wtile")
            nc.vector.memset(wtile, 0.25)
            wup = psum.tile([P, ccap], f32, tag="ps_g", name="wup")
            for _ in range(48):
                nc.tensor.matmul(wup[:, :P], wtile, wtile,
                                 start=True, stop=True, skip_group_check=True)

            def load_slab(src, nt):
                s = wpool.tile([P, KT, P], f16, tag="wslab", name="wslab")
                nc.sync.dma_start(s, src[nt])
                return s

            # ---- layer 1 + swiglu + rtne4, one 128-wide n-tile at a time ----
            for nt in range(NT):
                slab_g = load_slab(w1g_d, nt)
                slab_u = load_slab(w1u_d, nt)
                gps = psum.tile([P, ccap], f32, tag="ps_g", name="ps_g")
                ups = psum.tile([P, ccap], f32, tag="ps_u", name="ps_u")
                for k in range(KT):
                    nc.tensor.matmul(gps, slab_g[:, k, :], xts[:, k, :],
                                     start=(k == 0), stop=(k == KT - 1))
                    nc.tensor.matmul(ups, slab_u[:, k, :], xts[:, k, :],
                                     start=(k == 0), stop=(k == KT - 1))
                # swiglu: gate=min(G,7); up1=clip(U,-7,7)+1; x=gate*sig(1.702g)*up1
                gate = tmp.tile([P, ccap], f32, tag="t_gate", name="t_gate")
                nc.vector.tensor_scalar_min(gate, gps, 7.0)
                sig = tmp.tile([P, ccap], f32, tag="t_sig", name="t_sig")
                nc.scalar.activation(sig, gate, AF.Sigmoid, scale=1.702)
                up1 = tmp.tile([P, ccap], f32, tag="t_up", name="t_up")
                nc.vector.tensor_scalar(up1, ups, 1.0, -6.0, ALU.add, ALU.max)
                nc.vector.tensor_scalar_min(up1, up1, 8.0)
                nc.vector.tensor_mul(gate, gate, sig)          # gate*sig
                xv = tmp.tile([P, ccap], f32, tag="t_xv", name="t_xv")
                nc.vector.tensor_mul(xv, gate, up1)            # x = swiglu
                tv = tmp.tile([P, ccap], f32, tag="t_tv", name="t_tv")
                nc.vector.tensor_scalar_mul(tv, xv, VC)        # t = x*c
                nc.vector.tensor_sub(xv, tv, xv)               # d = t-x
                nc.vector.tensor_sub(interT[:, nt, :], tv, xv)  # rtne4 = t-d
            # layer-2 bias row: i == II lives at tile KT-1, partition II % P
            nc.vector.memset(interT[II % P : II % P + 1, KT - 1, :], 1.0)

            # ---- layer 2 + routing-weight scale (y stores batched 6-wide) ----
            YB = 6
            ysb = None
            for ht in range(NT):
                slab2 = load_slab(w2_d, ht)
                yps = psum.tile([P, ccap], f32, tag="ps_g", name="ps_g")
                for k in range(KT):
                    nc.tensor.matmul(yps, slab2[:, k, :], interT[:, k, :],
                                     start=(k == 0), stop=(k == KT - 1))
                if ht % YB == 0:
                    nyb = min(YB, NT - ht)
                    ysb = tmp.tile([P, YB, ccap], f16, tag="ysb", name="ysb")
                nc.vector.tensor_mul(ysb[:, ht % YB, :], yps, wrep)
                if ht % YB == nyb - 1:
                    nc.scalar.dma_start(
                        y_d[ht - nyb + 1 : ht + 1].rearrange("t p c -> p t c"),
                        ysb[:, :nyb, :],
                    )

    nc.finalize()
    return nc


def _stage(inputs):
    """Host-side routing + weight re-staging. Returns (nc, in_maps, assigns, T)."""
    hs = np.ascontiguousarray(np.asarray(inputs["hidden_states"], dtype=np.float32))
    ri = np.asarray(inputs["router_indices"]).astype(np.int64)
    rw = np.asarray(inputs["routing_weights"], dtype=np.float32)
    gup = np.asarray(inputs["gate_up_proj"], dtype=np.float32)
    gub = np.asarray(inputs["gate_up_proj_bias"], dtype=np.float32)
    dn = np.asarray(inputs["down_proj"], dtype=np.float32)
    dnb = np.asarray(inputs["down_proj_bias"], dtype=np.float32)

    T = hs.shape[0]
    topk = ri.shape[1]

    flat_e = ri.reshape(-1)
    order = np.argsort(flat_e, kind="stable")
    counts = np.bincount(flat_e, minlength=NE)
    starts = np.zeros(NE + 1, np.int64)
    starts[1:] = np.cumsum(counts)
    maxc = int(counts.max())
    # Each pass handles up to MAXTOK tokens per expert (seed-0 loads are ~142,
    # so this is a single pass; multiple passes only for pathological routing).
    npass = max(1, -(-maxc // MAXTOK))
    percap = -(-maxc // npass)
    ccap = max(32, -(-percap // 32) * 32)

    x_dq = _rtne4(hs).astype(np.float16)   # 4-sig-bit values: exact in fp16
    rw_flat = rw.reshape(-1)

    def stage_w(mat_t, bias):
        # mat_t: [K <= 2880, N <= 2880] contraction-major; -> tiled layout
        w = np.zeros((KP, NT * P), np.float16)
        w[: mat_t.shape[0], : mat_t.shape[1]] = mat_t
        w[H, : bias.shape[0]] = bias
        # [NT, P(k-partition), KT, P(n)] so each partition's slab bytes are
        # one contiguous HBM run
        return np.ascontiguousarray(
            w.reshape(KT, P, NT, P).transpose(2, 1, 0, 3)
        )

    passes, assigns = [], []
    weights = []
    for e in range(NE):
        weights.append((
            stage_w(gup[e, 0::2, :].T.astype(np.float16), gub[e, 0::2]),
            stage_w(gup[e, 1::2, :].T.astype(np.float16), gub[e, 1::2]),
            stage_w(dn[e].T.astype(np.float16), dnb[e]),
        ))
    for p in range(npass):
        in_maps, passigns = [], []
        for e in range(NE):
            a_all = order[starts[e] : starts[e + 1]]
            a = a_all[p * ccap : (p + 1) * ccap]
            toks = a // topk
            ce = len(a)
            passigns.append((a, toks))

            xt = np.zeros((KP, ccap), np.float16)
            xt[:H, :ce] = x_dq[toks].T
            xt[H, :] = np.float16(1.0)
            xt = np.ascontiguousarray(xt.reshape(KT, P, ccap).transpose(1, 0, 2))

            wr_rep = np.zeros((P, ccap), np.float32)
            wr_rep[:, :ce] = rw_flat[a][None, :]

            w1g, w1u, w2 = weights[e]
            in_maps.append(dict(xt=xt, w1g=w1g, w1u=w1u, w2=w2, wr=wr_rep))
        passes.append(in_maps)
        assigns.append(passigns)

    nc = _build(ccap)
    return nc, passes, assigns, T


def kernel(**inputs):
    nc, passes, assigns, T = _stage(inputs)
    out = np.zeros((T, H), np.float32)
    for in_maps, passigns in zip(passes, assigns):
        res = run_bass_kernel_spmd(nc, in_maps, list(range(NE)))
        for e in range(NE):
            a, toks = passigns[e]
            if len(a):
                yt = res.results[e]["y"].reshape(NT * P, -1)[:H, : len(a)]
                np.add.at(out, toks, yt.T.astype(np.float32))
    return out

